# revision 8
# baseline (speedup 1.0000x reference)
"""Expert-parallel sparse GLU (MoE) kernel for 8 TRN2 NeuronCores.

Problem: x[16384,1024] tokens pre-sorted by expert, 8 experts with equal
capacity 2048; per expert e:
    out_e = (gelu(x_e @ w1[e].T) * (x_e @ v1[e].T)) @ w2[e]

Sharding: expert parallelism — core e computes expert e on its 2048-token
slice. Zero inter-core communication.

Per-core schedule (all fp32 storage, float32r matmuls = full PE rate):
  - xT [H=1024, cap=2048] resident in SBUF as [128, 8, 2048]
  - two c-blocks of 1024 tokens; per block:
      Phase A: for each f-tile (128 of F=2048): x1T/x2T = w1T/v1T-tile.T @ xT
               accumulated over H in PSUM; GLU (ACT gelu + DVE mul) into
               hT [128, 16, 1024] SBUF
      Phase B: out[c,h'] accumulated over F in PSUM: lhsT = hT f-tiles,
               rhs = streamed w2 tiles [128, 512]
"""

import numpy as np

T, H, F, E = 16384, 1024, 2048, 8
CAP = T // E  # 2048 tokens per expert/core
P = 128
KO = H // P            # 8 h-subtiles
FO = F // P            # 16 f-tiles
NBLK = 2               # c-blocks
CBLK = CAP // NBLK     # 1024
NQ = CBLK // 512       # 2 q-chunks of 512 per block
NCS = CBLK // P        # 8 c-subtiles per block
NH2 = H // 512         # 2 output column halves

_CACHE = {}


def _build_nc(act="Gelu", reps=1, probe_scale=False):
    import concourse.tile as tile
    from concourse import bacc
    import concourse.mybir as mybir

    f32 = mybir.dt.float32
    f32r = mybir.dt.float32r
    Gelu = getattr(mybir.ActivationFunctionType, act)

    nc = bacc.Bacc("TRN2", target_bir_lowering=False, debug=False, num_devices=E)

    xt = nc.dram_tensor("xt", [H, CAP], f32r, kind="ExternalInput").ap()
    w1t = nc.dram_tensor("w1t", [H, F], f32r, kind="ExternalInput").ap()
    v1t = nc.dram_tensor("v1t", [H, F], f32r, kind="ExternalInput").ap()
    w2 = nc.dram_tensor("w2", [F, H], f32r, kind="ExternalInput").ap()
    out = nc.dram_tensor("out", [CAP, H], f32, kind="ExternalOutput").ap()

    xt3 = xt.rearrange("(ko p) c -> p ko c", p=P)    # [128, 8, 2048]
    w1t3 = w1t.rearrange("(ko p) f -> p ko f", p=P)  # [128, 8, 2048]
    v1t3 = v1t.rearrange("(ko p) f -> p ko f", p=P)

    with tile.TileContext(nc) as tc:
        with (
            tc.tile_pool(name="xtp", bufs=1) as xtp,
            tc.tile_pool(name="htp", bufs=1) as htp,
            tc.tile_pool(name="wap", bufs=3) as wap,
            tc.tile_pool(name="wbp", bufs=4) as wbp,
            tc.tile_pool(name="tmpp", bufs=3) as tmpp,
            tc.tile_pool(name="obp", bufs=4) as obp,
            tc.tile_pool(name="psp", bufs=8, space="PSUM") as psp,
        ):
          for _rep in range(reps):  # reps>1 only for steady-state timing
            # resident xT: load per-ko so first matmuls start early
            xts = xtp.tile([P, KO, CAP], f32r, name="xts")
            for ko in range(KO):
                nc.sync.dma_start(xts[:, ko, :], xt3[:, ko, :])

            # hT for one c-block: [f%128, f//128, c within block]
            hts = htp.tile([P, FO, CBLK], f32r, name="hts")

            for blk in range(NBLK):
                c0 = blk * CBLK
                # ---------------- Phase A: x1T/x2T + GLU -> hT ----------
                for fo in range(FO):
                    fsl = slice(fo * P, (fo + 1) * P)
                    w1s = wap.tile([P, KO, P], f32r, tag="w1s")
                    nc.sync.dma_start(w1s[:], w1t3[:, :, fsl])
                    v1s = wap.tile([P, KO, P], f32r, tag="v1s")
                    nc.sync.dma_start(v1s[:], v1t3[:, :, fsl])

                    x1p = [psp.tile([P, 512], f32, tag="ps", name=f"x1p{q}")
                           for q in range(NQ)]
                    x2p = [psp.tile([P, 512], f32, tag="ps", name=f"x2p{q}")
                           for q in range(NQ)]
                    for ko in range(KO):
                        st = dict(start=(ko == 0), stop=(ko == KO - 1))
                        w1k = w1s[:, ko, :]
                        v1k = v1s[:, ko, :]
                        for q in range(NQ):
                            xk = xts[:, ko, c0 + q * 512: c0 + (q + 1) * 512]
                            nc.tensor.matmul(x1p[q][:], w1k, xk, **st)
                        for q in range(NQ):
                            xk = xts[:, ko, c0 + q * 512: c0 + (q + 1) * 512]
                            nc.tensor.matmul(x2p[q][:], v1k, xk, **st)
                    for q in range(NQ):
                        gtmp = tmpp.tile([P, 512], f32)
                        nc.scalar.activation(gtmp[:], x1p[q][:], Gelu)
                        nc.vector.tensor_mul(
                            hts[:, fo, q * 512:(q + 1) * 512], gtmp[:], x2p[q][:]
                        )

                # ---------------- Phase B: out = hT.T @ w2 --------------
                for h2 in range(NH2):
                    hsl = slice(h2 * 512, (h2 + 1) * 512)
                    op = [psp.tile([P, 512], f32, tag="ps", name=f"op{cs}")
                          for cs in range(NCS)]
                    for fo in range(FO):
                        w2s = wbp.tile([P, 512], f32r, tag="w2s")
                        nc.sync.dma_start(w2s[:], w2[fo * P:(fo + 1) * P, hsl])
                        w2r = w2s[:]
                        st = dict(start=(fo == 0), stop=(fo == FO - 1))
                        for cs in range(NCS):
                            hk = hts[:, fo, cs * P:(cs + 1) * P]
                            nc.tensor.matmul(op[cs][:], hk, w2r, **st)
                    for cs in range(NCS):
                        ob = obp.tile([P, 512], f32)
                        if probe_scale and _rep == reps - 1:
                            nc.scalar.mul(ob[:], op[cs][:], 2.0)
                        else:
                            nc.vector.tensor_copy(ob[:], op[cs][:])
                        nc.sync.dma_start(
                            out[c0 + cs * P: c0 + (cs + 1) * P, hsl], ob[:]
                        )
    nc.finalize()  # bacc register allocation + codegen passes
    return nc


def _get_nc():
    if "nc" not in _CACHE:
        _CACHE["nc"] = _build_nc()
    return _CACHE["nc"]


def kernel(x, w1, v1, w2, expert_ids):
    """Full inputs in, full output out. expert_ids is ignored: tokens are
    pre-sorted with equal capacity T//E (the reference ignores it too)."""
    from concourse.bass_utils import run_bass_kernel_spmd

    nc = _get_nc()

    x = np.asarray(x, dtype=np.float32)
    w1 = np.asarray(w1, dtype=np.float32)
    v1 = np.asarray(v1, dtype=np.float32)
    w2 = np.asarray(w2, dtype=np.float32)

    in_maps = []
    for e in range(E):
        xs = x[e * CAP:(e + 1) * CAP]  # [cap, H]
        in_maps.append({
            "xt": np.ascontiguousarray(xs.T),           # [H, cap]
            "w1t": np.ascontiguousarray(w1[e].T),       # [H, F]
            "v1t": np.ascontiguousarray(v1[e].T),       # [H, F]
            "w2": np.ascontiguousarray(w2[e]),          # [F, H]
        })

    res = run_bass_kernel_spmd(nc, in_maps, core_ids=list(range(E)))
    outs = [res.results[e]["out"] for e in range(E)]
    return np.concatenate(outs, axis=0).astype(np.float32)
